# revision 8
# baseline (speedup 1.0000x reference)
# Trainium2 Bass kernel for masked causal attention
#   B=2, H=16, S=2048, D=64, bool attn_mask [B, S, S] + causal, softmax, @V.
#
# Sharding: 8 cores x 4 heads (cores 0-3 -> batch 0, cores 4-7 -> batch 1).
#
# Softmax numerator/denominator are computed unnormalized on device (ones-row
# in the PV lhsT gives the denominator); the final divide happens on HOST.
#
# Per (head, k-tile j of 128 keys), causal span q in [128j, 2048), the exp
# pipeline is split in two paths to balance ACT and DVE:
#  - k-tiles 0..3 ("path B"): ACT exp reads score chunks straight from PSUM
#    f32 (8 chunks/head of <=1024), then the 0/1 mask is applied
#    multiplicatively on fp16 (GPSIMD for j<2, DVE for j 2..3).
#  - k-tiles 4..15 ("path A"): DVE drains PSUM with a fused additive mask
#    bias (0 keep / -100 masked) into causal-packed fp16 group buffers; ACT
#    then exps each 4-k-tile group in ONE wide in-place instruction
#    (instruction-overhead-free compared to <=1024 PSUM-sourced chunks).
# PV accumulates [V | ones] @ p per 512-wide q-bank into f32 PSUM; banks are
# drained by DVE and DMA'd out as they complete. PV matmuls of head h are
# interleaved into head h+1's QK stream so the PE never head-blocks on ACT
# and the HAM clock gate stays open.

import numpy as np

B, H, S, D = 2, 16, 2048, 64
NCORES = 8
HPC = 4          # heads per core
P = 128
NKT = S // P     # 16 k-tiles
DP1 = D + 1      # 64 value rows + denominator ones-row
GS = 4           # k-tiles per wide-exp group (path A)
CHUNK = 1024

W = [S - P * j for j in range(NKT)]              # causal span of k-tile j
OFF = [0] * NKT                                   # causal-packed offsets
for j in range(1, NKT):
    OFF[j] = OFF[j - 1] + W[j - 1]
AT = OFF[-1] + W[-1]                              # 17408
BJ = 4                                            # k-tiles on path B
BW = OFF[BJ]                                      # 7424 path-B packed width
AOFF = [OFF[j] - OFF[BJ] for j in range(NKT)]     # path-A packed offsets
AW = AT - BW                                      # 9984
GOFF = {g: AOFF[g * GS] for g in (1, 2, 3)}
GW = {g: sum(W[g * GS:(g + 1) * GS]) for g in (1, 2, 3)}
GPSIMD_BJ = 3    # path-B k-tiles j < this get their mask mult on GPSIMD
# B/A interleaved emission order: keeps ACT (path-B exp) and DVE (path-A
# drain) concurrently busy instead of phase-separating within each head
J_ORDER = [0, 4, 5, 6, 1, 7, 8, 9, 2, 10, 11, 12, 3, 13, 14, 15]

_cache = {}


def chunks(j):
    """1024-grid-aligned chunks covering [128j, 2048)."""
    out, c = [], P * j
    while c < S:
        e = min(S, (c // CHUNK + 1) * CHUNK)
        out.append((c, e))
        c = e
    return out


def build_nc():
    import concourse.bacc as bacc
    import concourse.mybir as mybir
    import concourse.tile as tile
    from contextlib import ExitStack

    fp16 = mybir.dt.float16
    f32 = mybir.dt.float32
    Exp = mybir.ActivationFunctionType.Exp

    from concourse import library_config

    nc = bacc.Bacc("TRN2", target_bir_lowering=False, debug=False,
                   num_devices=NCORES)

    qt_d = nc.dram_tensor("qt", [HPC, D, S], fp16, kind="ExternalInput")
    kt_d = nc.dram_tensor("kt", [HPC, D, S], fp16, kind="ExternalInput")
    vp_d = nc.dram_tensor("vp", [HPC, P, NKT, DP1], fp16, kind="ExternalInput")
    m01_d = nc.dram_tensor("m01", [P, BW], fp16, kind="ExternalInput")
    mb_d = nc.dram_tensor("mb", [P, AW], fp16, kind="ExternalInput")
    out_d = nc.dram_tensor("outt", [HPC, DP1, S], f32, kind="ExternalOutput")

    with tile.TileContext(nc) as tc, ExitStack() as ctx:
        mk_pool = ctx.enter_context(tc.tile_pool(name="mk", bufs=1))
        qk_pool = ctx.enter_context(tc.tile_pool(name="qk", bufs=2))
        vp_pool = ctx.enter_context(tc.tile_pool(name="vpool", bufs=2))
        sp_pool = ctx.enter_context(tc.tile_pool(name="sp", bufs=2))
        pb_pool = ctx.enter_context(tc.tile_pool(name="pb", bufs=8))
        osb_pool = ctx.enter_context(tc.tile_pool(name="osb", bufs=4))
        warm_pool = ctx.enter_context(tc.tile_pool(name="warm", bufs=1))
        st_psum = ctx.enter_context(tc.tile_pool(name="st", bufs=2, space="PSUM"))
        o_psum = ctx.enter_context(tc.tile_pool(name="outp", bufs=1, space="PSUM"))

        nc.gpsimd.load_library(library_config.standard)

        # PE warm-up: dense back-to-back matmuls on zeros so the HAM clock
        # gate opens to 2.4 GHz before the real QK stream begins.
        wsb = warm_pool.tile([P, 512], fp16, tag="warm")
        nc.vector.memset(wsb[:], 0.0)
        wps = o_psum.tile([P, 512], f32, tag="outp0")
        for _ in range(12):
            nc.tensor.matmul(wps[:], lhsT=wsb[:, 0:128], rhs=wsb[:],
                             start=True, stop=True)

        def load_head(h):
            qt = qk_pool.tile([D, S], fp16, tag="qt")
            nc.sync.dma_start(qt[:], qt_d[h])
            kt = qk_pool.tile([D, S], fp16, tag="kt")
            nc.sync.dma_start(kt[:], kt_d[h])
            vp = vp_pool.tile([P, NKT, DP1], fp16, tag="vp")
            nc.sync.dma_start(vp[:], vp_d[h])
            return qt, kt, vp

        # Head 0 inputs first (unblocks the first QK), then the masks stream
        # in one causal-packed plane per k-tile behind it.
        head_tiles = {0: load_head(0)}
        m01_sb = mk_pool.tile([P, BW], fp16, tag="m01")
        mb_sb = mk_pool.tile([P, AW], fp16, tag="mb")
        for j in range(BJ):
            nc.sync.dma_start(m01_sb[:, OFF[j]:OFF[j] + W[j]],
                              m01_d[:, OFF[j]:OFF[j] + W[j]])
        for j in range(BJ, NKT):
            nc.sync.dma_start(mb_sb[:, AOFF[j]:AOFF[j] + W[j]],
                              mb_d[:, AOFF[j]:AOFF[j] + W[j]])

        def qk_units(h, qt, kt, vp, sp_tiles, pb_tiles):
            """One callable per (k-tile, chunk): QK MMs + exp pipeline."""
            def unit(j, c, e):
                def run():
                    w = e - c
                    st = st_psum.tile([P, CHUNK], f32, tag="st",
                                      name=f"st_h{h}j{j}")
                    for lo in range(0, w, 512):
                        wl = min(512, w - lo)
                        nc.tensor.matmul(
                            st[:, lo:lo + wl],
                            lhsT=kt[:, j * P:(j + 1) * P],
                            rhs=qt[:, c + lo:c + lo + wl],
                            start=True, stop=True)
                    if j < BJ:
                        # path B: chunk exp from PSUM, then 0/1 mask mult
                        pb = pb_pool.tile([P, CHUNK], fp16, tag="pb",
                                          name=f"pb_h{h}j{j}c{c}")
                        nc.scalar.activation(pb[:, :w], st[:, :w], Exp)
                        mo = OFF[j] + (c - P * j)
                        eng = nc.gpsimd if j < GPSIMD_BJ else nc.vector
                        eng.tensor_mul(pb[:, :w], pb[:, :w],
                                       m01_sb[:, mo:mo + w])
                        pb_tiles[(j, c)] = pb
                    else:
                        # path A: fused drain + additive mask bias
                        g = j // GS
                        if j % GS == 0 and c == P * j:
                            sp_tiles[g] = sp_pool.tile(
                                [P, GW[g]], fp16, tag=f"sp{g}",
                                name=f"sp_h{h}g{g}")
                        lo = AOFF[j] - GOFF[g] + (c - P * j)
                        nc.vector.tensor_add(sp_tiles[g][:, lo:lo + w],
                                             st[:, :w],
                                             mb_sb[:, AOFF[j] + (c - P * j):
                                                   AOFF[j] + (c - P * j) + w])
                        if j % GS == GS - 1 and e == S:
                            # one wide in-place exp for the whole group
                            nc.scalar.activation(sp_tiles[g][:], sp_tiles[g][:],
                                                 Exp)
                return run
            return [unit(j, c, e) for j in J_ORDER for c, e in chunks(j)]

        def pv_units(h, vp, sp_tiles, pb_tiles):
            """PV MMs (j ascending => per-bank start..stop order) plus the
            per-bank drain right after the bank's last MM."""
            outp = {}
            units = []

            def mk_mm(j, b):
                def run():
                    if b not in outp:
                        outp[b] = o_psum.tile([DP1, 512], f32, tag=f"outp{b}",
                                              name=f"outp_h{h}b{b}")
                    q0 = max(P * j, 512 * b)
                    q1 = 512 * (b + 1)
                    if j < BJ:
                        c = (q0 // CHUNK) * CHUNK if q0 >= CHUNK else P * j
                        rhs = pb_tiles[(j, c)][:, q0 - c:q1 - c]
                    else:
                        g = j // GS
                        lo = AOFF[j] - GOFF[g] + (q0 - P * j)
                        rhs = sp_tiles[g][:, lo:lo + (q1 - q0)]
                    nc.tensor.matmul(
                        outp[b][:, q0 - 512 * b:q1 - 512 * b],
                        lhsT=vp[:, j, :], rhs=rhs,
                        start=(j == 0),
                        stop=(j == min(4 * b + 3, NKT - 1)))
                return run

            def mk_drain(b):
                def run():
                    osb = osb_pool.tile([DP1, 512], f32, tag="osb",
                                        name=f"osb_h{h}b{b}")
                    nc.vector.tensor_copy(osb[:], outp[b][:])
                    nc.sync.dma_start(out_d[h, :, 512 * b:512 * (b + 1)],
                                      osb[:])
                return run

            for j in range(NKT):
                for b in range(j // 4, 4):
                    units.append(mk_mm(j, b))
                    if j == min(4 * b + 3, NKT - 1):
                        units.append(mk_drain(b))
            return units

        def interleave(qk, pv):
            """Emit QK units with pv callables spread between them."""
            done = 0
            for i, u in enumerate(qk):
                u()
                want = (i + 1) * len(pv) // len(qk)
                while done < want:
                    pv[done]()
                    done += 1
            while done < len(pv):
                pv[done]()
                done += 1

        prev_pv = []
        for h in range(HPC):
            qt, kt, vp = head_tiles.pop(h, None) or load_head(h)
            sp_tiles, pb_tiles = {}, {}
            interleave(qk_units(h, qt, kt, vp, sp_tiles, pb_tiles), prev_pv)
            prev_pv = pv_units(h, vp, sp_tiles, pb_tiles)
            if h + 1 < HPC:
                head_tiles[h + 1] = load_head(h + 1)
        for u in prev_pv:
            u()

    nc.compile()
    return nc


def prep_inputs(query, key, value, attn_mask):
    """Host-side layout prep (transposes/retiling/casts only) -> 8 in_maps."""
    query = np.asarray(query, dtype=np.float32)
    key = np.asarray(key, dtype=np.float32)
    value = np.asarray(value, dtype=np.float32)
    attn_mask = np.asarray(attn_mask).astype(bool)

    # fold the 1/sqrt(D)=0.125 softmax scale into Q
    qT = np.ascontiguousarray(
        (query * 0.125).transpose(0, 1, 3, 2)).astype(np.float16)
    kT = np.ascontiguousarray(key.transpose(0, 1, 3, 2)).astype(np.float16)

    vp = np.concatenate(
        [value, np.ones((B, H, S, 1), np.float32)], axis=3).astype(np.float16)
    # [B, H, S, 65] -> [B, H, 128, NKT, 65] (partition-contiguous tiles)
    vp = np.ascontiguousarray(
        vp.reshape(B, H, NKT, P, DP1).transpose(0, 1, 3, 2, 4))

    tril = np.tril(np.ones((S, S), dtype=bool))
    in_maps = []
    for b in range(B):
        m = (attn_mask[b] & tril)          # [q, k] True = keep
        mT = m.T                           # [k, q]
        # causal-packed masks: plane j = rows [128j,128j+128) of mT,
        # cols [128j, 2048). Path B (j<4): 0/1 multiplicative; path A
        # (j>=4): additive bias 0 keep / -100 masked.
        m01 = np.empty((P, BW), np.float16)
        for j in range(BJ):
            keep = mT[P * j:P * (j + 1), P * j:]
            m01[:, OFF[j]:OFF[j] + W[j]] = keep.astype(np.float16)
        mb = np.empty((P, AW), np.float16)
        for j in range(BJ, NKT):
            keep = mT[P * j:P * (j + 1), P * j:]
            mb[:, AOFF[j]:AOFF[j] + W[j]] = np.where(keep, np.float16(0.0),
                                                     np.float16(-100.0))
        for cl in range(NCORES // B):
            h0 = cl * HPC
            in_maps.append({
                "qt": np.ascontiguousarray(qT[b, h0:h0 + HPC]),
                "kt": np.ascontiguousarray(kT[b, h0:h0 + HPC]),
                "vp": np.ascontiguousarray(vp[b, h0:h0 + HPC]),
                "m01": m01,
                "mb": mb,
            })
    return in_maps


def run(query, key, value, attn_mask, trace=False, trace_cores=None):
    from concourse import bass_utils

    if "nc" not in _cache:
        _cache["nc"] = build_nc()
    nc = _cache["nc"]

    in_maps = prep_inputs(query, key, value, attn_mask)
    res = bass_utils.run_bass_kernel_spmd(
        nc, in_maps, core_ids=list(range(NCORES)),
        trace=trace, trace_cores=trace_cores)

    out = np.empty((B, H, S, D), np.float32)
    for c in range(NCORES):
        b = c // (NCORES // B)
        h0 = (c % (NCORES // B)) * HPC
        outt = res.results[c]["outt"]          # [HPC, 65, S]
        num = outt[:, 0:D, :]                  # [HPC, 64, S]
        den = outt[:, D:D + 1, :]              # [HPC, 1, S]
        out[b, h0:h0 + HPC] = (num / den).transpose(0, 2, 1)
    return out, res


def kernel(query, key, value, attn_mask):
    out, _ = run(query, key, value, attn_mask)
    return out


# revision 11
# speedup vs baseline: 1.0479x; 1.0479x over previous
# Trainium2 Bass kernel for masked causal attention
#   B=2, H=16, S=2048, D=64, bool attn_mask [B, S, S] + causal, softmax, @V.
#
# Sharding: 8 cores x 4 heads (cores 0-3 -> batch 0, cores 4-7 -> batch 1).
#
# Softmax numerator/denominator are computed unnormalized on device (ones-row
# in the PV lhsT gives the denominator); the final divide happens on HOST.
#
# Per (head, k-tile j of 128 keys), causal span q in [128j, 2048), the exp
# pipeline is split in two paths to balance ACT and DVE:
#  - k-tiles 0..3 ("path B"): ACT exp reads score chunks straight from PSUM
#    f32 (8 chunks/head of <=1024), then the 0/1 mask is applied
#    multiplicatively on fp16 (GPSIMD for j<2, DVE for j 2..3).
#  - k-tiles 4..15 ("path A"): DVE drains PSUM with a fused additive mask
#    bias (0 keep / -100 masked) into causal-packed fp16 group buffers; ACT
#    then exps each 4-k-tile group in ONE wide in-place instruction
#    (instruction-overhead-free compared to <=1024 PSUM-sourced chunks).
# PV accumulates [V | ones] @ p per 512-wide q-bank into f32 PSUM; banks are
# drained by DVE and DMA'd out as they complete. PV matmuls of head h are
# interleaved into head h+1's QK stream so the PE never head-blocks on ACT
# and the HAM clock gate stays open.

import numpy as np

B, H, S, D = 2, 16, 2048, 64
NCORES = 8
HPC = 4          # heads per core
P = 128
NKT = S // P     # 16 k-tiles
DP1 = D + 1      # 64 value rows + denominator ones-row
GS = 4           # k-tiles per wide-exp group (path A)
CHUNK = 1024

W = [S - P * j for j in range(NKT)]              # causal span of k-tile j
OFF = [0] * NKT                                   # causal-packed offsets
for j in range(1, NKT):
    OFF[j] = OFF[j - 1] + W[j - 1]
AT = OFF[-1] + W[-1]                              # 17408
BJ = 4                                            # k-tiles on path B
BW = OFF[BJ]                                      # 7424 path-B packed width
AOFF = [OFF[j] - OFF[BJ] for j in range(NKT)]     # path-A packed offsets
AW = AT - BW                                      # 9984
GOFF = {g: AOFF[g * GS] for g in (1, 2, 3)}
GW = {g: sum(W[g * GS:(g + 1) * GS]) for g in (1, 2, 3)}
GPSIMD_BJ = 2    # path-B k-tiles j < this get their mask mult on GPSIMD
import os
PAD_LDW = os.environ.get("ATTN_PAD", "1") == "1"
J_ORDER = list(range(NKT))

_cache = {}


def chunks(j):
    """1024-grid-aligned chunks covering [128j, 2048)."""
    out, c = [], P * j
    while c < S:
        e = min(S, (c // CHUNK + 1) * CHUNK)
        out.append((c, e))
        c = e
    return out


def build_nc():
    import concourse.bacc as bacc
    import concourse.mybir as mybir
    import concourse.tile as tile
    from contextlib import ExitStack

    fp16 = mybir.dt.float16
    f32 = mybir.dt.float32
    Exp = mybir.ActivationFunctionType.Exp

    from concourse import library_config

    nc = bacc.Bacc("TRN2", target_bir_lowering=False, debug=False,
                   num_devices=NCORES)

    qt_d = nc.dram_tensor("qt", [HPC, D, S], fp16, kind="ExternalInput")
    kt_d = nc.dram_tensor("kt", [HPC, D, S], fp16, kind="ExternalInput")
    vp_d = nc.dram_tensor("vp", [HPC, P, NKT, DP1], fp16, kind="ExternalInput")
    m01_d = nc.dram_tensor("m01", [P, BW], fp16, kind="ExternalInput")
    mb_d = nc.dram_tensor("mb", [P, AW], fp16, kind="ExternalInput")
    out_d = nc.dram_tensor("outt", [HPC, DP1, S], f32, kind="ExternalOutput")

    with tile.TileContext(nc) as tc, ExitStack() as ctx:
        mk_pool = ctx.enter_context(tc.tile_pool(name="mk", bufs=1))
        qk_pool = ctx.enter_context(tc.tile_pool(name="qk", bufs=2))
        vp_pool = ctx.enter_context(tc.tile_pool(name="vpool", bufs=2))
        sp_pool = ctx.enter_context(tc.tile_pool(name="sp", bufs=2))
        pb_pool = ctx.enter_context(tc.tile_pool(name="pb", bufs=6))
        osb_pool = ctx.enter_context(tc.tile_pool(name="osb", bufs=4))
        warm_pool = ctx.enter_context(tc.tile_pool(name="warm", bufs=1))
        st_psum = ctx.enter_context(tc.tile_pool(name="st", bufs=2, space="PSUM"))
        o_psum = ctx.enter_context(tc.tile_pool(name="outp", bufs=1, space="PSUM"))

        nc.gpsimd.load_library(library_config.standard)

        # PE warm-up: dense back-to-back matmuls on zeros so the HAM clock
        # gate opens to 2.4 GHz before the real QK stream begins.
        wsb = warm_pool.tile([P, 512], fp16, tag="warm")
        nc.vector.memset(wsb[:], 0.0)
        wps = o_psum.tile([P, 512], f32, tag="outp0")
        for _ in range(12):
            nc.tensor.matmul(wps[:], lhsT=wsb[:, 0:128], rhs=wsb[:],
                             start=True, stop=True)

        def load_head(h):
            qt = qk_pool.tile([D, S], fp16, tag="qt")
            nc.sync.dma_start(qt[:], qt_d[h])
            kt = qk_pool.tile([D, S], fp16, tag="kt")
            nc.sync.dma_start(kt[:], kt_d[h])
            vp = vp_pool.tile([P, NKT, DP1], fp16, tag="vp")
            nc.sync.dma_start(vp[:], vp_d[h])
            return qt, kt, vp

        # Head 0 inputs first (unblocks the first QK), then the masks stream
        # in one causal-packed plane per k-tile behind it.
        head_tiles = {0: load_head(0)}
        m01_sb = mk_pool.tile([P, BW], fp16, tag="m01")
        mb_sb = mk_pool.tile([P, AW], fp16, tag="mb")
        for j in range(BJ):
            nc.sync.dma_start(m01_sb[:, OFF[j]:OFF[j] + W[j]],
                              m01_d[:, OFF[j]:OFF[j] + W[j]])
        for j in range(BJ, NKT):
            nc.sync.dma_start(mb_sb[:, AOFF[j]:AOFF[j] + W[j]],
                              mb_d[:, AOFF[j]:AOFF[j] + W[j]])

        def qk_units(h, qt, kt, vp, sp_tiles, pb_tiles):
            """One callable per (k-tile, chunk): QK MMs + exp pipeline."""
            def unit(j, c, e):
                def run():
                    w = e - c
                    st = st_psum.tile([P, CHUNK], f32, tag="st",
                                      name=f"st_h{h}j{j}")
                    for lo in range(0, w, 512):
                        wl = min(512, w - lo)
                        nc.tensor.matmul(
                            st[:, lo:lo + wl],
                            lhsT=kt[:, j * P:(j + 1) * P],
                            rhs=qt[:, c + lo:c + lo + wl],
                            start=True, stop=True)
                    if PAD_LDW:
                        # HAM keep-alive: a row-disjoint LDWEIGHTS (rows
                        # 64-127, QK tiles use 0-63) occupies the PE during
                        # st-ring stalls so the clock gate stays at 8/8.
                        nc.tensor.ldweights(weights=wsb[64:128, 0:128],
                                            tile_position=(64, 0))
                    if j < BJ:
                        # path B: chunk exp from PSUM, then 0/1 mask mult
                        pb = pb_pool.tile([P, CHUNK], fp16, tag="pb",
                                          name=f"pb_h{h}j{j}c{c}")
                        nc.scalar.activation(pb[:, :w], st[:, :w], Exp)
                        mo = OFF[j] + (c - P * j)
                        eng = nc.gpsimd if j < GPSIMD_BJ else nc.vector
                        eng.tensor_mul(pb[:, :w], pb[:, :w],
                                       m01_sb[:, mo:mo + w])
                        pb_tiles[(j, c)] = pb
                    else:
                        # path A: fused drain + additive mask bias
                        g = j // GS
                        if j % GS == 0 and c == P * j:
                            sp_tiles[g] = sp_pool.tile(
                                [P, GW[g]], fp16, tag=f"sp{g}",
                                name=f"sp_h{h}g{g}")
                        lo = AOFF[j] - GOFF[g] + (c - P * j)
                        nc.vector.tensor_add(sp_tiles[g][:, lo:lo + w],
                                             st[:, :w],
                                             mb_sb[:, AOFF[j] + (c - P * j):
                                                   AOFF[j] + (c - P * j) + w])
                        if j % GS == GS - 1 and e == S:
                            # one wide in-place exp for the whole group
                            nc.scalar.activation(sp_tiles[g][:], sp_tiles[g][:],
                                                 Exp)
                return run
            return [unit(j, c, e) for j in J_ORDER for c, e in chunks(j)]

        def pv_units(h, vp, sp_tiles, pb_tiles):
            """PV MMs (j ascending => per-bank start..stop order) plus the
            per-bank drain right after the bank's last MM."""
            outp = {}
            units = []

            def mk_mm(j, b):
                def run():
                    if b not in outp:
                        outp[b] = o_psum.tile([DP1, 512], f32, tag=f"outp{b}",
                                              name=f"outp_h{h}b{b}")
                    q0 = max(P * j, 512 * b)
                    q1 = 512 * (b + 1)
                    if j < BJ:
                        c = (q0 // CHUNK) * CHUNK if q0 >= CHUNK else P * j
                        rhs = pb_tiles[(j, c)][:, q0 - c:q1 - c]
                    else:
                        g = j // GS
                        lo = AOFF[j] - GOFF[g] + (q0 - P * j)
                        rhs = sp_tiles[g][:, lo:lo + (q1 - q0)]
                    nc.tensor.matmul(
                        outp[b][:, q0 - 512 * b:q1 - 512 * b],
                        lhsT=vp[:, j, :], rhs=rhs,
                        start=(j == 0),
                        stop=(j == min(4 * b + 3, NKT - 1)))
                return run

            def mk_drain(b):
                def run():
                    osb = osb_pool.tile([DP1, 512], f32, tag="osb",
                                        name=f"osb_h{h}b{b}")
                    nc.vector.tensor_copy(osb[:], outp[b][:])
                    nc.sync.dma_start(out_d[h, :, 512 * b:512 * (b + 1)],
                                      osb[:])
                return run

            for j in range(NKT):
                for b in range(j // 4, 4):
                    units.append(mk_mm(j, b))
                    if j == min(4 * b + 3, NKT - 1):
                        units.append(mk_drain(b))
            return units

        def interleave(qk, pv):
            """Emit QK units with pv callables spread between them."""
            done = 0
            for i, u in enumerate(qk):
                u()
                want = (i + 1) * len(pv) // len(qk)
                while done < want:
                    pv[done]()
                    done += 1
            while done < len(pv):
                pv[done]()
                done += 1

        prev_pv = []
        for h in range(HPC):
            qt, kt, vp = head_tiles.pop(h, None) or load_head(h)
            sp_tiles, pb_tiles = {}, {}
            interleave(qk_units(h, qt, kt, vp, sp_tiles, pb_tiles), prev_pv)
            prev_pv = pv_units(h, vp, sp_tiles, pb_tiles)
            if h + 1 < HPC:
                head_tiles[h + 1] = load_head(h + 1)
        for u in prev_pv:
            u()

    nc.compile()
    return nc


def prep_inputs(query, key, value, attn_mask):
    """Host-side layout prep (transposes/retiling/casts only) -> 8 in_maps."""
    query = np.asarray(query, dtype=np.float32)
    key = np.asarray(key, dtype=np.float32)
    value = np.asarray(value, dtype=np.float32)
    attn_mask = np.asarray(attn_mask).astype(bool)

    # fold the 1/sqrt(D)=0.125 softmax scale into Q
    qT = np.ascontiguousarray(
        (query * 0.125).transpose(0, 1, 3, 2)).astype(np.float16)
    kT = np.ascontiguousarray(key.transpose(0, 1, 3, 2)).astype(np.float16)

    vp = np.concatenate(
        [value, np.ones((B, H, S, 1), np.float32)], axis=3).astype(np.float16)
    # [B, H, S, 65] -> [B, H, 128, NKT, 65] (partition-contiguous tiles)
    vp = np.ascontiguousarray(
        vp.reshape(B, H, NKT, P, DP1).transpose(0, 1, 3, 2, 4))

    tril = np.tril(np.ones((S, S), dtype=bool))
    in_maps = []
    for b in range(B):
        m = (attn_mask[b] & tril)          # [q, k] True = keep
        mT = m.T                           # [k, q]
        # causal-packed masks: plane j = rows [128j,128j+128) of mT,
        # cols [128j, 2048). Path B (j<4): 0/1 multiplicative; path A
        # (j>=4): additive bias 0 keep / -100 masked.
        m01 = np.empty((P, BW), np.float16)
        for j in range(BJ):
            keep = mT[P * j:P * (j + 1), P * j:]
            m01[:, OFF[j]:OFF[j] + W[j]] = keep.astype(np.float16)
        mb = np.empty((P, AW), np.float16)
        for j in range(BJ, NKT):
            keep = mT[P * j:P * (j + 1), P * j:]
            mb[:, AOFF[j]:AOFF[j] + W[j]] = np.where(keep, np.float16(0.0),
                                                     np.float16(-100.0))
        for cl in range(NCORES // B):
            h0 = cl * HPC
            in_maps.append({
                "qt": np.ascontiguousarray(qT[b, h0:h0 + HPC]),
                "kt": np.ascontiguousarray(kT[b, h0:h0 + HPC]),
                "vp": np.ascontiguousarray(vp[b, h0:h0 + HPC]),
                "m01": m01,
                "mb": mb,
            })
    return in_maps


def run(query, key, value, attn_mask, trace=False, trace_cores=None):
    from concourse import bass_utils

    if "nc" not in _cache:
        _cache["nc"] = build_nc()
    nc = _cache["nc"]

    in_maps = prep_inputs(query, key, value, attn_mask)
    res = bass_utils.run_bass_kernel_spmd(
        nc, in_maps, core_ids=list(range(NCORES)),
        trace=trace, trace_cores=trace_cores)

    out = np.empty((B, H, S, D), np.float32)
    for c in range(NCORES):
        b = c // (NCORES // B)
        h0 = (c % (NCORES // B)) * HPC
        outt = res.results[c]["outt"]          # [HPC, 65, S]
        num = outt[:, 0:D, :]                  # [HPC, 64, S]
        den = outt[:, D:D + 1, :]              # [HPC, 1, S]
        out[b, h0:h0 + HPC] = (num / den).transpose(0, 2, 1)
    return out, res


def kernel(query, key, value, attn_mask):
    out, _ = run(query, key, value, attn_mask)
    return out
